# revision 29
# baseline (speedup 1.0000x reference)
"""Trainium2 Bass kernel for nn_AttentionDecoder (8-core tensor-parallel).

Key observations about the reference model:
  - The attention softmax is over a size-1 axis, so its weights are exactly 1.0
    and ctx = X.sum(axis=1) is constant across all decode steps; the whole
    attention branch (Wa/ba/Wh/bh/Wo/bo) is dead code.
  - The embedding contribution to the LSTM gates, emb @ W_ih[:E], is a fixed
    linear map of the token -> precomputed on the host into a [V, 4H] table so
    each step only needs a 64-row gather (indirect DMA), not a matmul. The
    logits-side table emb @ Wout is concatenated into the same table so one
    gather per step serves both injects.
  - comb @ Wout = (e + ctx@Wlc) @ Wout + h @ (Wlh@Wout): the second factor M is
    host-precomputed, so per-step logits are one PSUM accumulation whose
    e-dependent part prefires before the h-state AllGather lands.
  - All sigmoids become tanh (sigma(x) = (tanh(x/2)+1)/2) by pre-scaling gate
    columns on the host and keeping doubled state (C2=2c, H2=2h with W_hh and
    Wlh pre-halved), so the LSTM needs a single ACT table load per step.

Distribution (8 cores, tensor parallel; B=64 stays whole):
  - H=1024 sharded 128/core (gate-interleaved); cell state stays sharded;
    h shards AllGathered (bf16) each step.
  - Wout/M V-sharded 1250/core; per-tile (512/512/226) local argmax candidates
    combined via a tiny [64,8] AllGather; log-softmax epilogue is lagged one
    step so its sumexp rides the next stats exchange.
  - h0 init is computed replicated (full Wh0 on every core) so setup needs no
    init h-exchange; a tiny warmup AllGather at kernel start absorbs the
    first-collective setup cost under the X load.
"""

import os
import sys

sys.path.insert(0, "/opt/trn_rl_repo")

import numpy as np
import ml_dtypes

BF = ml_dtypes.bfloat16

B, N, C, E, H, V, T = 64, 196, 512, 512, 1024, 10000, 20
NC_ = 8                    # cores
HS = H // NC_              # 128 h-shard
VS = V // NC_              # 1250 vocab shard
BS = B // NC_              # 8 batch rows per core (X reduction only)
NSTEP = T - 1              # 19 decode steps
START_IDX = 1
XROWS = BS * N             # 1568 rows of [C] per core
XTILES = (XROWS + 127) // 128  # 13
EWC = E + VS               # 1762 combined gather row
NTILES = [(0, 512), (512, 512), (1024, 226)]


def _build(nc, tile, mybir, bass, n_steps=NSTEP):
    f32 = mybir.dt.float32
    bf16 = mybir.dt.bfloat16
    i32 = mybir.dt.int32
    u32 = mybir.dt.uint32
    AF = mybir.ActivationFunctionType
    ALU = mybir.AluOpType
    AX = mybir.AxisListType
    from concourse.masks import make_identity
    from concourse.tile_rust import add_dep_helper

    def dep(a, b):
        add_dep_helper(a.ins, b.ins, reason="order after stats dma")

    # ---- DRAM parameters ----
    xk = nc.dram_tensor("xk", [XROWS, C], f32, kind="ExternalInput")
    bsel = nc.dram_tensor("bsel", [128, XTILES * BS], f32, kind="ExternalInput")
    w1 = nc.dram_tensor("w1", [128, 12 * 512], bf16, kind="ExternalInput")
    wlc = nc.dram_tensor("wlc", [128, 4 * 512], bf16, kind="ExternalInput")
    wout = nc.dram_tensor("wout", [128, 4 * VS], bf16, kind="ExternalInput")
    wm = nc.dram_tensor("wm", [128, 8 * VS], bf16, kind="ExternalInput")
    wc0 = nc.dram_tensor("wc0", [128, 512], bf16, kind="ExternalInput")
    wh0f = nc.dram_tensor("wh0f", [128, 4 * 1024], bf16, kind="ExternalInput")
    ew = nc.dram_tensor("ew", [V, E], bf16, kind="ExternalInput")
    ewo = nc.dram_tensor("ewo", [V, VS], bf16, kind="ExternalInput")
    coff3 = nc.dram_tensor("coff3", [B, 3], f32, kind="ExternalInput")
    out_ext = nc.dram_tensor("out", [n_steps, B, VS], f32, kind="ExternalOutput")

    RG = [list(range(NC_))]

    with tile.TileContext(nc) as tc:
        with (
            tc.tile_pool(name="wpool", bufs=1) as wpool,
            tc.tile_pool(name="sb", bufs=2) as sb,
            tc.tile_pool(name="lg", bufs=3) as lg,
            tc.tile_pool(name="psum", bufs=1, space="PSUM") as pp,
            tc.tile_pool(name="psT", bufs=2, space="PSUM") as ppT,
            tc.tile_pool(name="dram", bufs=2, space="DRAM") as dram,
        ):
            # ---- persistent SBUF ----
            w1_sb = wpool.tile([128, 12 * 512], bf16)
            wlc_sb = wpool.tile([128, 4 * 512], bf16)
            wout_sb = wpool.tile([128, 4 * VS], bf16)
            wm_sb = wpool.tile([128, 8 * VS], bf16)
            wc0_sb = wpool.tile([128, 512], bf16)
            wh0f_sb = wpool.tile([128, 4 * 1024], bf16)
            bsel_sb = wpool.tile([128, XTILES * BS], f32)
            id_f = wpool.tile([128, 128], f32)
            id_b = wpool.tile([128, 128], bf16)
            base0 = wpool.tile([B, 1280], bf16)
            lcT = wpool.tile([128, 4 * 64], bf16)
            ctxT = wpool.tile([128, 4 * 64], bf16)
            xmT = wpool.tile([128, 4 * 64], bf16)
            coff_sb = wpool.tile([B, 3], f32)
            lc_sb = wpool.tile([B, C], f32)
            maxc_all = wpool.tile([B, 8 * n_steps], f32)
            sec_all = wpool.tile([B, 8 * n_steps], bf16)
            gm_all = wpool.tile([B, n_steps], f32)

            # warmup collective: absorb first-CC setup cost under the X load
            warm_sb = sb.tile([1, 4], f32, tag="warm")
            nc.vector.memset(warm_sb[:], 0.0)
            warm_in = dram.tile([1, 4], f32, name="warm_in")
            warm_out = dram.tile(
                [NC_, 1, 4], f32, name="warm_out", addr_space="Shared"
            )
            nc.gpsimd.dma_start(warm_in[:], warm_sb[:])
            nc.gpsimd.collective_compute(
                "AllGather", ALU.bypass, replica_groups=RG,
                ins=[warm_in[:].opt()], outs=[warm_out[:].opt()],
            )

            # small/critical DMAs first so the ctx reduction can start early
            nc.sync.dma_start(bsel_sb[:], bsel[:])
            nc.sync.dma_start(coff_sb[:], coff3[:])

            # X tiles next (big DMA, spread queues)
            xts = []
            for i in range(XTILES):
                xt = sb.tile([128, C], f32, tag=f"xt{i % 4}", name=f"xt{i}")
                rows = min(128, XROWS - i * 128)
                eng = [nc.scalar, nc.gpsimd, nc.sync][i % 3]
                eng.dma_start(xt[:rows, :], xk[i * 128 : i * 128 + rows, :])
                xts.append((xt, rows))

            nc.gpsimd.dma_start(w1_sb[:], w1[:])
            nc.gpsimd.dma_start(wlc_sb[:], wlc[:])
            nc.scalar.dma_start(wout_sb[:], wout[:])
            nc.scalar.dma_start(wm_sb[:], wm[:])
            nc.sync.dma_start(wc0_sb[:], wc0[:])
            nc.sync.dma_start(wh0f_sb[:], wh0f[:])
            make_identity(nc, id_f[:])
            make_identity(nc, id_b[:])

            # ---- ctx = X.sum(axis=1) ----
            ps_xs = pp.tile([BS, C], f32, tag="pg")
            for i, (xt, rows) in enumerate(xts):
                nc.tensor.matmul(
                    out=ps_xs[:],
                    lhsT=bsel_sb[:rows, i * BS : (i + 1) * BS],
                    rhs=xt[:rows, :],
                    start=(i == 0),
                    stop=(i == XTILES - 1),
                )
            xs_sb = sb.tile([BS, C], f32, tag="xs")
            nc.vector.tensor_copy(xs_sb[:], ps_xs[:])

            bxs_in = dram.tile([BS, C], f32, name="bxs_in")
            bxs_out = dram.tile(
                [NC_, BS, C], f32, name="bxs_out", addr_space="Shared"
            )
            i_bxs = nc.gpsimd.dma_start(bxs_in[:], xs_sb[:])
            nc.gpsimd.collective_compute(
                "AllGather", ALU.bypass, replica_groups=RG,
                ins=[bxs_in[:].opt()], outs=[bxs_out[:].opt()],
            )
            xs_all = sb.tile([B, C], f32, tag="xsall")
            nc.gpsimd.dma_start(xs_all[:], bxs_out[:].rearrange("j b c -> (j b) c"))

            # keep the PE clock warm through the bxs AllGather so the whole
            # lc/base0/h0 setup matmul chain runs at full rate
            junk0_ps = pp.tile([B, 512], f32, tag="junk", name="junk_setup")
            prevj0 = None
            for q in range(12):
                ij0 = nc.tensor.matmul(
                    out=junk0_ps[:], lhsT=id_b[:, 0:64],
                    rhs=w1_sb[:, 0:512], start=True, stop=True,
                )
                if prevj0 is None:
                    dep(ij0, i_bxs)
                else:
                    dep(ij0, prevj0)
                prevj0 = ij0

            for cc in range(4):
                pT = ppT.tile([128, 64], f32, tag="pT", name=f"pctx{cc}")
                nc.tensor.transpose(
                    out=pT[:], in_=xs_all[:, cc * 128 : (cc + 1) * 128],
                    identity=id_f[:64, :64],
                )
                nc.vector.tensor_copy(ctxT[:, cc * 64 : (cc + 1) * 64], pT[:])
                nc.vector.tensor_scalar_mul(
                    xmT[:, cc * 64 : (cc + 1) * 64], pT[:], 1.0 / N
                )

            # lc_const = ctx @ Wlc  (biases are all zero in this model)
            ps_lc = pp.tile([B, C], f32, tag="pg", name="ps_lc")
            for cc in range(4):
                nc.tensor.matmul(
                    out=ps_lc[:], lhsT=ctxT[:, cc * 64 : (cc + 1) * 64],
                    rhs=wlc_sb[:, cc * 512 : (cc + 1) * 512],
                    start=(cc == 0), stop=(cc == 3),
                )
            nc.vector.tensor_copy(lc_sb[:], ps_lc[:])
            for cc in range(4):
                pT2 = ppT.tile([128, 64], f32, tag="pT", name=f"plc{cc}")
                nc.tensor.transpose(
                    out=pT2[:], in_=lc_sb[:, cc * 128 : (cc + 1) * 128],
                    identity=id_f[:64, :64],
                )
                nc.vector.tensor_copy(lcT[:, cc * 64 : (cc + 1) * 64], pT2[:])
            for ntt, (noff, nsz) in enumerate(NTILES):
                ps_b0 = pp.tile([B, nsz], f32, tag=f"pl{ntt}", name=f"psb0_{ntt}")
                for cc in range(4):
                    nc.tensor.matmul(
                        out=ps_b0[:],
                        lhsT=lcT[:, cc * 64 : (cc + 1) * 64],
                        rhs=wout_sb[:, cc * VS + noff : cc * VS + noff + nsz],
                        start=(cc == 0), stop=(cc == 3),
                    )
                nc.vector.tensor_copy(base0[:, noff : noff + nsz], ps_b0[:])

            # ---- C2_0 = 2*tanh(Xm@Wc0) (local shard) ----
            ps_c0 = pp.tile([B, HS], f32, tag="pcb")
            for cc in range(4):
                nc.tensor.matmul(
                    out=ps_c0[:], lhsT=xmT[:, cc * 64 : (cc + 1) * 64],
                    rhs=wc0_sb[:, cc * 128 : (cc + 1) * 128],
                    start=(cc == 0), stop=(cc == 3),
                )
            th0 = sb.tile([B, HS], f32, tag="th0")
            nc.scalar.activation(th0[:], ps_c0[:], AF.Tanh)
            c2 = sb.tile([B, HS], f32, tag="c", name="c_init")
            nc.vector.tensor_scalar_mul(c2[:], th0[:], 2.0)

            # ---- H2_0 = 2*tanh(Xm@Wh0) computed FULL on every core (no AG) ----
            ps_h0a = pp.tile([B, 512], f32, tag="pl0", name="ps_h0a")
            ps_h0b = pp.tile([B, 512], f32, tag="pl1", name="ps_h0b")
            for cc in range(4):
                nc.tensor.matmul(
                    out=ps_h0a[:], lhsT=xmT[:, cc * 64 : (cc + 1) * 64],
                    rhs=wh0f_sb[:, cc * 1024 : cc * 1024 + 512],
                    start=(cc == 0), stop=(cc == 3),
                )
            for cc in range(4):
                nc.tensor.matmul(
                    out=ps_h0b[:], lhsT=xmT[:, cc * 64 : (cc + 1) * 64],
                    rhs=wh0f_sb[:, cc * 1024 + 512 : (cc + 1) * 1024],
                    start=(cc == 0), stop=(cc == 3),
                )
            h0f = sb.tile([B, 1024], f32, tag="h0f")
            nc.scalar.activation(h0f[:, 0:512], ps_h0a[:], AF.Tanh)
            nc.scalar.activation(h0f[:, 512:1024], ps_h0b[:], AF.Tanh)
            h2T_all = sb.tile([128, NC_ * 64], bf16, tag="h2T", name="h2Tinit")
            for j in range(NC_):
                pTh = ppT.tile([128, 64], f32, tag="pT", name=f"ph0T{j}")
                nc.tensor.transpose(
                    out=pTh[:], in_=h0f[:, j * 128 : (j + 1) * 128],
                    identity=id_f[:64, :64],
                )
                nc.vector.tensor_scalar_mul(
                    h2T_all[:, j * 64 : (j + 1) * 64], pTh[:], 2.0
                )

            # ---- h-exchange: H2 [64,128] -> AllGather -> transposed unpack ----
            HB = HS * B
            def exchange_h(h2_tile, step):
                bh_in = dram.tile([HB], bf16, tag="bh_in", name=f"bh_in{step}")
                bh_out = dram.tile(
                    [NC_, HB], bf16, tag="bh_out", name=f"bh_out{step}",
                    addr_space="Shared",
                )
                nc.sync.dma_start(
                    bh_in[:].rearrange("(b c) -> b c", c=HS), h2_tile[:]
                )
                nc.gpsimd.collective_compute(
                    "AllGather", ALU.bypass, replica_groups=RG,
                    ins=[bh_in[:].opt()], outs=[bh_out[:].opt()],
                )
                h2T = sb.tile([128, NC_ * 64], bf16, tag="h2T", name=f"h2T{step}")
                src = bh_out[:].rearrange("j (b c) -> j b c", c=HS)
                i_tp = nc.scalar.dma_start_transpose(
                    h2T[:, 0 : 4 * 64],
                    src[0:4].rearrange("j b c -> (j b) c"),
                )
                nc.sync.dma_start_transpose(
                    h2T[:, 4 * 64 : 8 * 64],
                    src[4:8].rearrange("j b c -> (j b) c"),
                )
                return h2T, i_tp

            se_bf = sb.tile([B, 1], bf16, tag="sebf", name="se_init")
            nc.vector.memset(se_bf[:], 0.0)

            tok = sb.tile([B, 1], i32, tag="tok", name="tok_init")
            nc.gpsimd.memset(tok[:], START_IDX)

            logits_tiles = []
            # ---- decode steps ----
            for t in range(n_steps):
                # two gathers from the combined table: the gates slice first
                # (it gates the LSTM), the logits slice rides behind it
                ew_row = sb.tile(
                    [B, E], bf16, tag="ewrow", bufs=3, name=f"ewrow{t}"
                )
                nc.gpsimd.indirect_dma_start(
                    out=ew_row[:], out_offset=None, in_=ew[:],
                    in_offset=bass.IndirectOffsetOnAxis(ap=tok[:, :1], axis=0),
                )
                ewo_row = sb.tile(
                    [B, VS], bf16, tag="eworow", bufs=3, name=f"eworow{t}"
                )
                nc.gpsimd.indirect_dma_start(
                    out=ewo_row[:], out_offset=None, in_=ewo[:],
                    in_offset=bass.IndirectOffsetOnAxis(ap=tok[:, :1], axis=0),
                )

                # gates matmuls (h-part usable as soon as h2T_all exists)
                ps_g = pp.tile([B, 512], f32, tag="pg", name=f"psg{t}")
                for j in range(12):
                    lhsT = (
                        h2T_all[:, j * 64 : (j + 1) * 64]
                        if j < 8
                        else ctxT[:, (j - 8) * 64 : (j - 7) * 64]
                    )
                    nc.tensor.matmul(
                        out=ps_g[:], lhsT=lhsT, rhs=w1_sb[:, j * 512 : (j + 1) * 512],
                        start=(j == 0), stop=False,
                    )
                nc.tensor.matmul(
                    out=ps_g[:], lhsT=id_b[:64, :64], rhs=ew_row[:],
                    start=False, stop=True,
                )
                # all-tanh LSTM: th = tanh(gates) (i,f,o pre-halved on host)
                th = sb.tile([B, 512], f32, tag="th", name=f"th{t}")
                nc.scalar.activation(th[:], ps_g[:], AF.Tanh)
                ti, tf = th[:, 0:128], th[:, 128:256]
                tg, to = th[:, 256:384], th[:, 384:512]
                aa = sb.tile([B, HS], f32, tag="aa", name=f"aa{t}")
                nc.vector.scalar_tensor_tensor(
                    out=aa[:], in0=tf, scalar=1.0, in1=c2[:],
                    op0=ALU.add, op1=ALU.mult,
                )  # (tf'+1)*C2 = 4*sigma_f*c
                bb = sb.tile([B, HS], f32, tag="bb", name=f"bb{t}")
                nc.vector.scalar_tensor_tensor(
                    out=bb[:], in0=ti, scalar=1.0, in1=tg,
                    op0=ALU.add, op1=ALU.mult,
                )  # (ti'+1)*tg = 2*sigma_i*tg
                c2 = sb.tile([B, HS], f32, tag="c", name=f"c{t}")
                nc.vector.scalar_tensor_tensor(
                    out=c2[:], in0=aa[:], scalar=0.5, in1=bb[:],
                    op0=ALU.mult, op1=ALU.add,
                )  # C2' = 0.5*aa + bb = 2*c'
                tc2 = sb.tile([B, HS], f32, tag="tc2", name=f"tc2_{t}")
                nc.scalar.activation(tc2[:], c2[:], AF.Tanh, scale=0.5)
                h2 = sb.tile([B, HS], bf16, tag="hbf", name=f"h{t}")
                nc.vector.scalar_tensor_tensor(
                    out=h2[:], in0=to, scalar=1.0, in1=tc2[:],
                    op0=ALU.add, op1=ALU.mult,
                )  # H2' = (to'+1)*tanh(c') = 2h'

                # logits base: base0 + EWOUT[tok] injected via identity matmuls
                ps_l = []
                for ntt, (noff, nsz) in enumerate(NTILES):
                    pl = pp.tile([B, nsz], f32, tag=f"pl{ntt}", name=f"psl{t}_{ntt}")
                    nc.tensor.matmul(
                        out=pl[:], lhsT=id_b[:64, :64],
                        rhs=base0[:, noff : noff + nsz],
                        start=True, stop=False,
                    )
                    nc.tensor.matmul(
                        out=pl[:], lhsT=id_b[:64, :64],
                        rhs=ewo_row[:, noff : noff + nsz],
                        start=False, stop=False,
                    )
                    ps_l.append(pl)

                # keep the PE clock warm through the h-exchange window:
                # dead matmuls into a scratch PSUM bank (never read). The
                # tensor engine drops to half rate after a few us idle and
                # the logits burst otherwise runs entirely at low p-state.
                junk_ps = pp.tile([B, 512], f32, tag="junk", name=f"junk{t}")
                prevj = None
                for q in range(12):
                    ij = nc.tensor.matmul(
                        out=junk_ps[:], lhsT=id_b[:, 0:64],
                        rhs=w1_sb[:, 0:512], start=True, stop=True,
                    )
                    if prevj is not None:
                        dep(ij, prevj)
                    prevj = ij

                # exchange h; then logits += H2 @ M
                h2T_all, i_tp = exchange_h(h2, t)
                # second warm pulse right when the exchange result lands
                for q in range(4):
                    ij = nc.tensor.matmul(
                        out=junk_ps[:], lhsT=id_b[:, 0:64],
                        rhs=w1_sb[:, 0:512], start=True, stop=True,
                    )
                    dep(ij, prevj if q else i_tp)
                    prevj = ij
                first_wm = None
                for ntt, (noff, nsz) in enumerate(NTILES):
                    for j in range(8):
                        imm = nc.tensor.matmul(
                            out=ps_l[ntt][:],
                            lhsT=h2T_all[:, j * 64 : (j + 1) * 64],
                            rhs=wm_sb[:, j * VS + noff : j * VS + noff + nsz],
                            start=False, stop=(j == 7),
                        )
                        if first_wm is None:
                            first_wm = imm
                dep(first_wm, prevj)

                # per-tile local argmax candidates straight off PSUM, so each
                # tile's scan overlaps the next tile's matmuls
                mx, ix = [], []
                for ntt in range(3):
                    m_t = sb.tile([B, 8], f32, tag=f"mx{ntt}", name=f"mx{t}_{ntt}")
                    nc.vector.max(out=m_t[:], in_=ps_l[ntt][:])
                    i_t = sb.tile([B, 8], u32, tag=f"ix{ntt}", name=f"ix{t}_{ntt}")
                    nc.vector.max_index(i_t[:], m_t[:], ps_l[ntt][:])
                    mx.append(m_t)
                    ix.append(i_t)

                # stats row: [m0 m1 m2 gi0 gi1 gi2 se_prev (se_last)]
                stats = sb.tile([B, 8], f32, tag="stats", name=f"stats{t}")
                for ntt in range(3):
                    nc.vector.tensor_copy(stats[:, ntt : ntt + 1], mx[ntt][:, :1])
                    nc.vector.tensor_scalar_add(
                        stats[:, 3 + ntt : 4 + ntt], ix[ntt][:, :1],
                        coff_sb[:, ntt : ntt + 1],
                    )
                nc.vector.tensor_copy(stats[:, 6:7], se_bf[:])

                # local sumexp (lagged; rides the NEXT stats exchange). On
                # all but the last step this runs AFTER the stats DMA fires
                # (dep-pinned below) so it never delays the exchange.
                def se_block(t=t):
                    out = []
                    mloc = sb.tile([B, 1], f32, tag="mloc", name=f"mloc{t}")
                    out.append(nc.vector.tensor_tensor(
                        out=mloc[:], in0=mx[0][:, :1], in1=mx[1][:, :1],
                        op=ALU.max,
                    ))
                    negmax = sb.tile([B, 1], f32, tag="negmax", name=f"nm{t}")
                    out.append(nc.vector.scalar_tensor_tensor(
                        out=negmax[:], in0=mloc[:], scalar=-1.0,
                        in1=mx[2][:, :1], op0=ALU.bypass, op1=ALU.max,
                    ))
                    out.append(nc.vector.tensor_scalar_mul(
                        negmax[:], negmax[:], -1.0
                    ))
                    logits = lg.tile([B, 1280], bf16, tag="lgt", name=f"lg{t}")
                    for ntt, (noff, nsz) in enumerate(NTILES):
                        out.append(nc.vector.tensor_copy(
                            logits[:, noff : noff + nsz], ps_l[ntt][:]
                        ))
                    exp_trash = sb.tile([B, VS], bf16, tag="expt", name=f"ex{t}")
                    se_val = sb.tile([B, 1], f32, tag="seval", name=f"sev{t}")
                    nc.scalar.activation(
                        exp_trash[:], logits[:, :VS], AF.Exp,
                        bias=negmax[:, :1], accum_out=se_val[:],
                    )
                    se_new = sb.tile([B, 1], bf16, tag="sebf", name=f"sebf{t}")
                    out.append(nc.vector.tensor_copy(se_new[:], se_val[:]))
                    return logits, se_new, out

                bs_in = dram.tile([B, 8], f32, tag="bs_in", name=f"bs_in{t}")
                bs_out = dram.tile(
                    [NC_, B, 8], f32, tag="bs_out", name=f"bs_out{t}",
                    addr_space="Shared",
                )
                if t == n_steps - 1:
                    # last step: its own sumexp goes in col 7 of the same AG
                    logits, se_bf, _ = se_block()
                    nc.vector.tensor_copy(stats[:, 7:8], se_bf[:])
                    nc.gpsimd.dma_start(bs_in[:], stats[:])
                else:
                    i_bsin = nc.gpsimd.dma_start(bs_in[:], stats[:])
                    logits, se_bf, se_insts = se_block()
                    for si in se_insts:
                        dep(si, i_bsin)
                nc.gpsimd.collective_compute(
                    "AllGather", ALU.bypass, replica_groups=RG,
                    ins=[bs_in[:].opt()], outs=[bs_out[:].opt()],
                )
                statsg = sb.tile([B, NC_ * 8], f32, tag="statsg", name=f"statsg{t}")
                nc.gpsimd.dma_start(
                    statsg[:].rearrange("b (j s) -> b j s", j=NC_),
                    bs_out[:].rearrange("j b s -> b j s"),
                )
                sview = statsg[:].rearrange("b (j s) -> b j s", s=8)
                m3d, i3d = sview[:, :, 0:3], sview[:, :, 3:6]
                if t > 0:
                    nc.vector.tensor_copy(
                        sec_all[:, (t - 1) * 8 : t * 8], sview[:, :, 6]
                    )
                if t == n_steps - 1:
                    nc.vector.tensor_copy(
                        sec_all[:, t * 8 : (t + 1) * 8], sview[:, :, 7]
                    )

                # per-core max (for the lagged log-softmax) and global max
                nc.vector.tensor_reduce(
                    out=maxc_all[:, t * 8 : (t + 1) * 8], in_=m3d,
                    axis=AX.X, op=ALU.max,
                )
                gmax = sb.tile([B, 1], f32, tag="gmax", name=f"gmax{t}")
                nc.vector.tensor_reduce(
                    out=gmax[:], in_=maxc_all[:, t * 8 : (t + 1) * 8],
                    axis=AX.X, op=ALU.max,
                )
                nc.vector.tensor_copy(gm_all[:, t : t + 1], gmax[:])
                if t < n_steps - 1:
                    ismax = sb.tile([B, 24], f32, tag="ismax", name=f"ismax{t}")
                    iv = ismax[:].rearrange("b (j s) -> b j s", s=3)
                    nc.vector.tensor_scalar(
                        iv, m3d, gmax[:, :1], None, op0=ALU.is_ge
                    )
                    cand = sb.tile([B, 24], f32, tag="cand", name=f"cand{t}")
                    nc.vector.tensor_tensor(
                        out=cand[:].rearrange("b (j s) -> b j s", s=3),
                        in0=i3d, in1=iv, op=ALU.mult,
                    )
                    tokf = sb.tile([B, 1], f32, tag="tokf", name=f"tokf{t}")
                    nc.vector.tensor_reduce(
                        out=tokf[:], in_=cand[:], axis=AX.X, op=ALU.max
                    )
                    tok = sb.tile([B, 1], i32, tag="tok", name=f"tok{t}")
                    nc.vector.tensor_copy(tok[:], tokf[:])
                logits_tiles.append(logits)
                if t > 0:
                    emit_lp(t - 1)

                # lagged log-softmax for step t-1 (sumexp arrived in this AG)
                def emit_lp(tt):
                    z8p = sb.tile([B, 8], f32, tag="z8p", name=f"z8p{tt}")
                    nc.vector.tensor_scalar(
                        z8p[:], maxc_all[:, tt * 8 : (tt + 1) * 8],
                        gm_all[:, tt : tt + 1], None, op0=ALU.subtract,
                    )
                    ez8p = sb.tile([B, 8], f32, tag="ez8p", name=f"ez8p{tt}")
                    nc.scalar.activation(ez8p[:], z8p[:], AF.Exp)
                    wz8p = sb.tile([B, 8], f32, tag="wz8p", name=f"wz8p{tt}")
                    nc.vector.tensor_mul(
                        wz8p[:], ez8p[:], sec_all[:, tt * 8 : (tt + 1) * 8]
                    )
                    ssp = sb.tile([B, 1], f32, tag="ssp", name=f"ssp{tt}")
                    nc.vector.tensor_reduce(
                        out=ssp[:], in_=wz8p[:], axis=AX.X, op=ALU.add
                    )
                    lnp = sb.tile([B, 1], f32, tag="lnp", name=f"lnp{tt}")
                    nc.scalar.activation(lnp[:], ssp[:], AF.Ln)
                    lsep = sb.tile([B, 1], f32, tag="lsep", name=f"lsep{tt}")
                    nc.vector.tensor_add(lsep[:], lnp[:], gm_all[:, tt : tt + 1])
                    lpp = sb.tile([B, VS], f32, tag=f"lpp{tt % 2}", name=f"lpp{tt}")
                    nc.vector.tensor_scalar(
                        lpp[:], logits_tiles[tt][:, :VS], lsep[:, :1],
                        None, op0=ALU.subtract,
                    )
                    # final step's output goes to sync (no h2 staging left to
                    # protect) so the two last 320KB stores drain in parallel
                    eng_lpp = nc.sync if tt == n_steps - 1 else nc.scalar
                    eng_lpp.dma_start(out_ext[tt], lpp[:])

            # tail: last step's log-softmax only
            emit_lp(n_steps - 1)

    nc.finalize()
    return nc


def _host_prep(inputs):
    X = np.asarray(inputs["X"], np.float32)
    emb = np.asarray(inputs["emb"], np.float32)
    W_ih = np.asarray(inputs["W_ih"], np.float32)
    b_ih = np.asarray(inputs["b_ih"], np.float32)
    W_hh = np.asarray(inputs["W_hh"], np.float32)
    b_hh = np.asarray(inputs["b_hh"], np.float32)
    Wlh = np.asarray(inputs["Wlh"], np.float32)
    Wlc = np.asarray(inputs["Wlc"], np.float32)
    Wout = np.asarray(inputs["Wout"], np.float32)
    Wc0 = np.asarray(inputs["Wc0"], np.float32)
    Wh0 = np.asarray(inputs["Wh0"], np.float32)

    EW = emb @ W_ih[:E] + b_ih + b_hh          # [V, 4H] token gate table
    M = 0.5 * (Wlh @ Wout)                      # [H, V]; 0.5 absorbs H2=2h

    # gate input scaling for the all-tanh LSTM: i,f,o halved; W_hh rows halved
    gsc = np.concatenate(
        [np.full(H, 0.5 if g != 2 else 1.0, np.float32) for g in range(4)]
    )
    W1full = np.concatenate([0.5 * W_hh, W_ih[E:]], axis=0) * gsc  # [1536, 4H]
    EW = (EW * gsc).astype(BF)

    def slab(w):
        k = w.shape[0] // 128
        return np.ascontiguousarray(
            w.reshape(k, 128, w.shape[1]).transpose(1, 0, 2).reshape(128, -1)
        )

    bselh = np.zeros((XTILES, 128, BS), np.float32)
    for r in range(XROWS):
        bselh[r // 128, r % 128, r // N] = 1.0
    bsel_l = np.ascontiguousarray(bselh.transpose(1, 0, 2).reshape(128, -1))
    wlc_l = slab(Wlc.astype(BF))
    wh0f_l = slab(Wh0.astype(BF))

    in_maps = []
    for k in range(NC_):
        cols = np.concatenate(
            [np.arange(g * H + k * HS, g * H + (k + 1) * HS) for g in range(4)]
        )
        coff3_k = np.zeros((B, 3), np.float32)
        for ntt, (noff, _) in enumerate(NTILES):
            coff3_k[:, ntt] = k * VS + noff
        in_maps.append(
            dict(
                xk=np.ascontiguousarray(X[k * BS : (k + 1) * BS].reshape(XROWS, C)),
                bsel=bsel_l,
                w1=slab(W1full[:, cols].astype(BF)),
                wlc=wlc_l,
                wout=slab(Wout[:, k * VS : (k + 1) * VS].astype(BF)),
                wm=slab(M[:, k * VS : (k + 1) * VS].astype(BF)),
                wc0=slab(Wc0[:, k * HS : (k + 1) * HS].astype(BF)),
                wh0f=wh0f_l,
                ew=np.ascontiguousarray(EW[:, cols]),
                ewo=np.ascontiguousarray(
                    (emb @ Wout[:, k * VS : (k + 1) * VS]).astype(BF)
                ),
                coff3=coff3_k,
            )
        )
    return in_maps


def kernel(**inputs) -> np.ndarray:
    import concourse.bass as bass
    import concourse.bacc as bacc
    import concourse.mybir as mybir
    import concourse.tile as tile
    from concourse.bass_utils import run_bass_kernel_spmd

    nc = bacc.Bacc("TRN2", target_bir_lowering=False, debug=False, num_devices=NC_)
    _build(nc, tile, mybir, bass)
    in_maps = _host_prep(inputs)
    res = run_bass_kernel_spmd(nc, in_maps, core_ids=list(range(NC_)))

    out = np.zeros((B, T, V), np.float32)
    out[:, 0, START_IDX] = 1.0
    for k in range(NC_):
        out[:, 1:, k * VS : (k + 1) * VS] = res.results[k]["out"].transpose(1, 0, 2)
    return out
